# revision 44
# baseline (speedup 1.0000x reference)
"""Time-varying FIR (AllZeroDigitalFilter) on 8 TRN2 NeuronCores — v12.
~90.2us vs the 130us v3.1 baseline.

Structure (per core: 2 sequences x 8 chunks x 126 frame-rows):
  C[k, i'] = sum_j h[k, j] * x[k*80 + i' - j + pad],  i' in [0,160)
  y[k*80+i] = w1[i]*C_{k+1}[i] + w0[i]*C_k[80+i]

Key mechanism (the big win over v3.1): the free dim is INTERLEAVED —
col = u*8 + c (chunk innermost). The DVE tap-mult's x operand is then
a contiguous fp16 slice and the h-broadcast operand has innermost AP
[1, 8] (the 160-sample repeat is a stride-0 MIDDLE dim), so
tensor_tensor qualifies for the 2x_1p DVE perf mode (checked on the
innermost AP dim only: 2-byte dtype, step +-1, >=2 elems, 4B-aligned)
— 2 elem/lane/cycle, ~700ns/tap vs ~1.5us at 1x in the v3.1 layout.

Division of labor:
  DVE: 45 taps/seq as 2x-mode multi-tap mults (9 quads + 1 double +
       7 singles; extra taps ride a middle AP dim: stride -8 on x,
       +16 on h, amortizing the 58-cycle DVE init; the small-issue
       taper at the end lets the PE drain its quad-burst backlog
       before the pe_d handoff)
  ACT: 5 taps/seq as 8 chunk-wise Copy-with-scale products (strided
       step-8 APs, ~711ns each on HW — strided ScalarE ops cost ~40%
       more than contiguous; the cost model does not show this)
  PE : accumulates every plane into PSUM via identity-stationary
       matmuls (windows 480|480|320, ~541ns/plane issue rate; ldw-opt
       dedups the identity LDWEIGHTS); also applies the +1 partition
       shift (Vs = SH.T @ V) on the lower 640 cols only
  DVE tail: V = psum*rr; y = V_hi + Vs_psum (fp16 out, host casts)
GpSimd: only DMA issue; its tensor ops halve concurrent DVE
throughput (shared SBUF port).

Measured HW facts this schedule is built around:
  - per-queue DMA: ~45GB/s, ~0.8us serial issue per DMA, transfers
    from one queue overlap each other -> fan the gating loads (xa0,
    hg) out as 9 pieces over the 3 DMA-capable queues (sync, scalar
    = ACT, gpsimd); DVE compute starts ~14us (7.5us of that is fixed
    preamble)
  - PE matmul streams at 1 col/cycle with ~zero per-instr overhead
    when pipelined; PSUM-read TTs (V/blend) are 1x mode
  - seq boundary: PE may not write PSUM while DVE reads it, so PE
    restarts only after blend-0 (y_s); emitting V-0/blend-0 around
    the first seq-1 quad minimizes the exposed serialization
  - device clock throttles some runs ~1.2x; verify surprising
    timings with a rerun

Hard-won rules encoded here (from v3.1):
  - PE must never write PSUM (any bank) while DVE reads PSUM: PE is
    gated on v_s / y_s around the V and blend phases.
  - Completion semaphores ride on the last real engine instruction.
  - DMA completion semaphores: shared counters are safe only with
    full-sum thresholds (wait covers ALL DMAs that inc the counter).
  - Output staged [125, 640] fp16, de-interleaved + cast on host.

Sharding: pure data parallel across batch, 2 sequences per core.
"""

import sys

for p in ("/opt/trn_rl_repo", "/root/.axon_site/_ro/trn_rl_repo"):
    if p not in sys.path:
        sys.path.append(p)

import numpy as np
import concourse.bass as bass
import concourse.mybir as mybir
from concourse.ap import AP
from concourse.bass_utils import run_bass_kernel_spmd


def _enable_ldw_opt():
    """Dedup identical LDWEIGHTS (walrus --enable-ldw-opt): saves ~40us of
    redundant identity reloads on the PE. Idempotent monkeypatch."""
    from concourse import bass_utils as _bu

    if getattr(_bu.run_command, "_ldw_opt_patched", False):
        return
    _orig = _bu.run_command

    def _patched(cmd, **kw):
        cmd = [
            "--enable-ldw-opt=true" if c == "--enable-ldw-opt=false" else c
            for c in cmd
        ]
        return _orig(cmd, **kw)

    _patched._ldw_opt_patched = True
    _bu.run_command = _patched


_enable_ldw_opt()

B, T = 16, 80000
P, D = 80, 50
N = T // P
NCORES = 8
S = B // NCORES
FO = 125
NC = 8
CS = 212
PAD = D - 1 + P  # 129
GF = NC * 160  # 1280
XF = NC * CS  # 1696
WIN = [(0, 512), (512, 512), (1024, 256)]

F16 = mybir.dt.float16
FP32 = mybir.dt.float32

ND = 45  # DVE taps per seq
NA = 5  # ACT taps per seq: 45..49
ACT_TAPS = list(range(ND, D))
NA16 = NA * 16
# DVE issues per seq: 9 quads (taps 0..35), 1 double (36..37), then 7
# singles — the long taper lets the PE (0.54us/plane vs quad bursts of
# 4, ~2us behind after the quads) fully drain its backlog before the
# end-of-seq pe_d handoff.
NQ = 9
NISS = NQ + 8


def issue_planes(d):
    """planes (1-based l values) covered by issue d"""
    if d < NQ:
        return list(range(4 * d + 1, 4 * d + 5))
    if d == NQ:
        return [4 * NQ + 1, 4 * NQ + 2]
    return [4 * NQ + 2 + (d - NQ)]


_nc_cache = {}


def plane_schedule():
    """PE consume order. D-planes land in quad/double bursts; A-planes
    every ~5.7us; A-ready biased +2us."""

    def t_d(l):
        if l <= 4 * NQ:
            return 2729.0 * ((l + 3) // 4)
        if l <= 4 * NQ + 2:
            return 2729.0 * NQ + 1402.0
        return 2729.0 * NQ + 1402.0 + 826.0 * (l - 4 * NQ - 2)

    ev = [("D", k, t_d(k + 1)) for k in range(ND)]
    ev += [("A", k, 5700.0 * (k + 1) + 1000.0) for k in range(NA)]
    ev.sort(key=lambda e: e[2])
    return [(kind, k) for kind, k, _ in ev]


def build_nc():
    if "nc" in _nc_cache:
        return _nc_cache["nc"]
    nc = bass.Bass()
    xa_ext = nc.declare_dram_parameter("xae", [S, 126, XF], F16, isOutput=False)
    hg_ext = nc.declare_dram_parameter("hg", [126, D * 16], F16, isOutput=False)
    hg32_ext = nc.declare_dram_parameter("hg32", [126, NA16], FP32, isOutput=False)
    rr_ext = nc.declare_dram_parameter("rr", [128, 160], F16, isOutput=False)
    eye_ext = nc.declare_dram_parameter("eye", [126, 126], F16, isOutput=False)
    shm_ext = nc.declare_dram_parameter("shm", [126, 126], F16, isOutput=False)
    out_ext = nc.declare_dram_parameter("out", [S, FO, GF // 2], F16, isOutput=True)

    from contextlib import ExitStack

    with ExitStack() as _ctx:
        ec = _ctx.enter_context
        xa0 = ec(nc.sbuf_tensor([126, XF], F16))
        xa1 = ec(nc.sbuf_tensor([126, XF], F16))
        hgx = ec(nc.sbuf_tensor([126, D * 16], F16))
        hgx32 = ec(nc.sbuf_tensor([126, NA16], FP32))
        rrt = ec(nc.sbuf_tensor([128, 160], F16))
        eye = ec(nc.sbuf_tensor([126, 126], F16))
        shm = ec(nc.sbuf_tensor([126, 126], F16))
        dpb = [ec(nc.sbuf_tensor(f"dpl{i}", [126, 4 * GF], F16)) for i in range(6)]
        apb = [ec(nc.sbuf_tensor(f"apl{i}", [126, GF], F16)) for i in range(6)]
        vt0 = ec(nc.sbuf_tensor([126, GF], F16))
        vt1 = ec(nc.sbuf_tensor([126, GF], F16))
        yy0 = ec(nc.sbuf_tensor([125, GF // 2], F16))
        yy1 = ec(nc.sbuf_tensor([125, GF // 2], F16))
        ps0 = ec(nc.psum_tensor("ps0", [126, GF], FP32))
        ps1 = ec(nc.psum_tensor("ps1", [126, GF], FP32))
        xin0 = ec(nc.semaphore("xin0"))
        xin1 = ec(nc.semaphore("xin1"))
        o_s2 = ec(nc.semaphore("o_s2"))
        cin = ec(nc.semaphore("cin"))
        hgin = ec(nc.semaphore("hgin"))
        pein = ec(nc.semaphore("pein"))
        dp_s = ec(nc.semaphore("dp_s"))
        dc_s = ec(nc.semaphore("dc_s"))
        ap_s = ec(nc.semaphore("ap_s"))
        ac_s = ec(nc.semaphore("ac_s"))
        pe_d = ec(nc.semaphore("pe_d"))
        v_s = ec(nc.semaphore("v_s"))
        vsr = ec(nc.semaphore("vsr"))
        y_s = ec(nc.semaphore("y_s"))
        o_s = ec(nc.semaphore("o_s"))
        o_sg = ec(nc.semaphore("o_sg"))
        scr = ec(nc.sbuf_tensor([1, 8], F16))
        block = ec(nc.Block())

        xa = [xa0, xa1]
        xin = [xin0, xin1]
        vt = [vt0, vt1]
        yy = [yy0, yy1]
        psall = [ps0, ps1]

        def xslice(s, j):
            # contiguous interleaved window: col u*8+c, u from D-1-j
            o = (D - 1 - j) * NC
            return xa[s][:, o : o + GF]

        def xmulti(s, j, n):
            # taps j..j+n-1 as middle dim (stride -8): n contiguous
            # 1280-windows
            return AP(
                tensor=xa[s][:].tensor,
                offset=(D - 1 - j) * NC,
                ap=[[XF, 126], [-NC, n], [1, GF]],
            )

        def hb(s, j):
            # h[cFO+k, j] at free (u, c): innermost [1, 8] step-1 (2x
            # eligible), 160-repeat as stride-0 middle dim
            return AP(
                tensor=hgx[:].tensor,
                offset=j * 16 + s * NC,
                ap=[[D * 16, 126], [0, 160], [1, NC]],
            )

        def hbmulti(s, j, n):
            return AP(
                tensor=hgx[:].tensor,
                offset=j * 16 + s * NC,
                ap=[[D * 16, 126], [16, n], [0, 160], [1, NC]],
            )

        def xact(s, j, c):
            return AP(
                tensor=xa[s][:].tensor,
                offset=(D - 1 - j) * NC + c,
                ap=[[XF, 126], [NC, 160]],
            )

        def rrb(o, ln):
            return AP(
                tensor=rrt[:].tensor,
                offset=o // NC,
                ap=[[160, 126], [1, ln // NC], [0, NC]],
            )

        # input DMA fan-out (only sync/scalar/gpsimd queues can DMA).
        # Small pieces pipeline within a queue (~0.8us issue, ~45GB/s
        # per transfer, transfers overlap), so 2 xa0 pieces + 1 hg piece
        # per queue beats one big DMA per queue.
        @block.sync
        def _(sync):
            sync.dma_start(xa[0][84:105, :], xa_ext[0, 84:105, :]).then_inc(
                xin0, 16
            )
            sync.dma_start(xa[0][105:126, :], xa_ext[0, 105:126, :]).then_inc(
                xin0, 16
            )
            sync.dma_start(hgx[84:126, :], hg_ext[84:126, :]).then_inc(hgin, 16)
            sync.dma_start(eye[:], eye_ext[:]).then_inc(pein, 16)
            sync.dma_start(shm[:], shm_ext[:]).then_inc(pein, 16)
            sync.dma_start(xa[1][0:63, :], xa_ext[1, 0:63, :]).then_inc(xin1, 16)
            sync.dma_start(xa[1][63:126, :], xa_ext[1, 63:126, :]).then_inc(
                xin1, 16
            )
            sync.wait_ge(y_s, 1)
            sync.dma_start(out_ext[0, :, :], yy[0][:]).then_inc(o_s, 16)
            sync.wait_ge(y_s, 2)
            sync.dma_start(out_ext[1, :, 0:240], yy[1][:, 0:240]).then_inc(o_s, 16)
            sync.wait_ge(o_s, 32)
            sync.wait_ge(o_s2, 16)
            sync.wait_ge(o_sg, 16)

        @block.gpsimd
        def _(gpsimd):
            gpsimd.dma_start(xa[0][42:63, :], xa_ext[0, 42:63, :]).then_inc(
                xin0, 16
            )
            gpsimd.dma_start(xa[0][63:84, :], xa_ext[0, 63:84, :]).then_inc(
                xin0, 16
            )
            gpsimd.dma_start(hgx[42:84, :], hg_ext[42:84, :]).then_inc(hgin, 16)
            gpsimd.wait_ge(y_s, 2)
            gpsimd.dma_start(
                out_ext[1, :, 240:480], yy[1][:, 240:480]
            ).then_inc(o_sg, 16)

        def emit_vmult(vector, s):
            if s == 0:
                vector.wait_ge(cin, 32)
            vector.wait_ge(pe_d, s + 1)
            inst = vector.tensor_tensor(
                out=vt[s][:],
                in0=psall[s][:],
                in1=rrb(0, GF),
                op=mybir.AluOpType.mult,
            )
            inst.then_inc(v_s, 1)

        def emit_yblend(vector, s):
            # y[q, (i,c)] = V[q, (80+i,c)] + Vs[q, (i,c)]; Vs lives in
            # psum cols 0:640 (the DVE read spans bank 0 + half of 1)
            vector.wait_ge(vsr, s + 1)
            inst = vector.tensor_tensor(
                out=yy[s][:],
                in0=vt[s][0:125, 640:1280],
                in1=psall[s][0:125, 0:640],
                op=mybir.AluOpType.add,
            )
            inst.then_inc(y_s, 1)

        @block.vector
        def _(vector):
            vector.wait_ge(hgin, 48)
            for s in range(S):
                vector.wait_ge(xin[s], 96 if s == 0 else 32)
                for d in range(NISS):
                    i_glob = s * NISS + d
                    if i_glob >= 6:
                        vector.wait_ge(dc_s, i_glob - 5)
                    buf = dpb[i_glob % 6]
                    pl = issue_planes(d)
                    j = pl[0] - 1
                    if len(pl) == 1:
                        inst = vector.tensor_tensor(
                            out=buf[:, 0:GF],
                            in0=hb(s, j),
                            in1=xslice(s, j),
                            op=mybir.AluOpType.mult,
                        )
                    else:
                        inst = vector.tensor_tensor(
                            out=buf[:, 0 : len(pl) * GF],
                            in0=hbmulti(s, j, len(pl)),
                            in1=xmulti(s, j, len(pl)),
                            op=mybir.AluOpType.mult,
                        )
                    inst.then_inc(dp_s, len(pl))
                    if s == 1 and d == 0:
                        # seq-0 V after the first seq-1 quad (pe_d=1
                        # lands during that quad: no DVE stall), and
                        # blend-0 right behind it (~0.6us vsr stall)
                        # so y_s unblocks the PE's seq-1 asap — the
                        # pipeline end is PE-consumption-bound
                        emit_vmult(vector, 0)
                        emit_yblend(vector, 0)
            emit_vmult(vector, 1)
            emit_yblend(vector, 1)

        @block.tensor
        def _(tensor):
            tensor.wait_ge(pein, 32)
            sched = plane_schedule()
            nplanes = ND + NA
            for s in range(S):
                if s > 0:
                    # PE may not touch PSUM while DVE reads it (V/blend of
                    # the previous sequence)
                    tensor.wait_ge(y_s, s)
                done = 0
                for kind, k in sched:
                    if kind == "D":
                        l = k + 1  # plane index within seq, 1..45
                        g = s * ND + l
                        tensor.wait_ge(dp_s, g)
                        if l <= 4 * NQ:
                            d, off = (l - 1) // 4, (l - 1) % 4
                        elif l <= 4 * NQ + 2:
                            d, off = NQ, l - 4 * NQ - 1
                        else:
                            d, off = NQ + (l - 4 * NQ - 2), 0
                        last_of_issue = l == issue_planes(d)[-1]
                        src = dpb[(s * NISS + d) % 6]
                        base = off * GF
                    else:
                        g = s * NA + k + 1
                        tensor.wait_ge(ap_s, g)
                        src = apb[(g - 1) % 6]
                        base = 0
                    first = done == 0
                    done += 1
                    last = done == nplanes
                    for w, (o, ln) in enumerate(WIN):
                        inst = tensor.matmul(
                            psall[s][:, o : o + ln],
                            eye[:],
                            src[:, base + o : base + o + ln],
                            start=first,
                            stop=last,
                            skip_group_check=True,
                        )
                    if done == nplanes:
                        inst.then_inc(pe_d, 1)
                    elif kind == "D" and last_of_issue:
                        inst.then_inc(dc_s, 1)
                    elif kind == "A" and g % 2 == 0:
                        inst.then_inc(ac_s, 1)
                # Vs = SH.T @ V (the +1 partition shift) on the lower 640
                # cols only (all the blend reads), reusing this sequence's
                # psum banks after the V-mult consumed them
                tensor.wait_ge(v_s, s + 1)
                tensor.matmul(
                    psall[s][:, 0:512],
                    shm[:],
                    vt[s][:, 0:512],
                    start=True,
                    stop=True,
                    skip_group_check=True,
                )
                inst = tensor.matmul(
                    psall[s][:, 512:640],
                    shm[:],
                    vt[s][:, 512:640],
                    start=True,
                    stop=True,
                    skip_group_check=True,
                )
                inst.then_inc(vsr, 1)

        @block.scalar
        def _(scalar):
            scalar.dma_start(xa[0][0:21, :], xa_ext[0, 0:21, :]).then_inc(
                xin0, 16
            )
            scalar.dma_start(xa[0][21:42, :], xa_ext[0, 21:42, :]).then_inc(
                xin0, 16
            )
            scalar.dma_start(hgx[0:42, :], hg_ext[0:42, :]).then_inc(hgin, 16)
            scalar.dma_start(rrt[:], rr_ext[:]).then_inc(cin, 16)
            scalar.dma_start(hgx32[:], hg32_ext[:]).then_inc(cin, 16)
            # dummy activation pulls the ~1.3us ACT_TABLE_LOAD ahead of
            # the first real tap product
            scalar.activation(
                scr[:, 4:8], scr[:, 0:4], mybir.ActivationFunctionType.Copy
            )
            scalar.wait_ge(cin, 32)
            for s in range(S):
                scalar.wait_ge(xin[s], 96 if s == 0 else 32)
                for kk, j in enumerate(ACT_TAPS):
                    g = s * NA + kk + 1
                    if g > 6:
                        scalar.wait_ge(ac_s, (g - 6 + 1) // 2)
                    for c in range(NC):
                        inst = scalar.activation(
                            AP(
                                tensor=apb[(g - 1) % 6][:].tensor,
                                offset=c,
                                ap=[[GF, 126], [NC, 160]],
                            ),
                            xact(s, j, c),
                            mybir.ActivationFunctionType.Copy,
                            scale=hgx32[
                                :, kk * 16 + s * NC + c : kk * 16 + s * NC + c + 1
                            ],
                        )
                    inst.then_inc(ap_s, 1)
            scalar.wait_ge(y_s, 2)
            scalar.dma_start(out_ext[1, :, 480:640], yy[1][:, 480:640]).then_inc(
                o_s2, 16
            )

    _nc_cache["nc"] = nc
    return nc


def _prep_core_inputs(x, h):
    x = np.ascontiguousarray(x, dtype=np.float32)
    h = np.ascontiguousarray(h, dtype=np.float32)
    TPX = N * P + CS + 4
    xp = np.zeros((B, TPX), np.float16)
    xp[:, PAD : PAD + T] = x.astype(np.float16)
    # host im2col, interleaved: xae[b, q, u*8+c] = xp[b, (c*FO + q)*P + u]
    idx_q = (np.arange(126)[:, None, None] * P + np.arange(CS)[None, :, None]
             + np.arange(NC)[None, None, :] * FO * P)  # [126, CS, NC]
    xae_all = xp[:, idx_q.reshape(-1)].reshape(B, 126, NC * CS)
    hpad = np.concatenate([h, h[:, -1:, :]], axis=1)
    w1 = (np.arange(P, dtype=np.float32) / P).astype(np.float16)
    w0 = (1.0 - np.arange(P, dtype=np.float32) / P).astype(np.float16)
    rr = np.ascontiguousarray(
        np.broadcast_to(np.concatenate([w1, w0])[None, :], (128, 160))
    )
    eye = np.eye(126, dtype=np.float16)
    shm = np.zeros((126, 126), np.float16)
    shm[np.arange(1, 126), np.arange(125)] = 1.0  # SH[k', m] = 1 iff k' == m+1
    in_maps = []
    acols = np.concatenate([np.arange(j * 16, j * 16 + 16) for j in ACT_TAPS])
    for core in range(NCORES):
        sl = slice(core * S, (core + 1) * S)
        hc = hpad[sl]
        hg32 = np.zeros((126, D * 16), np.float32)
        for s in range(S):
            for c in range(NC):
                blk = hc[s, c * FO : c * FO + 126, :]
                hg32[:, s * NC + c :: 16] = blk
        in_maps.append(
            {
                "xae": np.ascontiguousarray(xae_all[sl]),
                "hg": hg32.astype(np.float16),
                "hg32": np.ascontiguousarray(hg32[:, acols]),
                "rr": rr,
                "eye": eye,
                "shm": shm,
            }
        )
    return in_maps


def _unstage(res):
    outs = []
    for c in range(NCORES):
        o = res.results[c]["out"]  # [S, 125, 80*8] fp16, col = i*8+c
        o = o.reshape(S, FO, P, NC).transpose(0, 3, 1, 2).reshape(S, T)
        outs.append(o)
    return np.ascontiguousarray(
        np.concatenate(outs, axis=0), dtype=np.float32
    )


def kernel(x, h, **kw):
    nc = build_nc()
    in_maps = _prep_core_inputs(x, h)
    res = run_bass_kernel_spmd(nc, in_maps, core_ids=list(range(NCORES)), **kw)
    return _unstage(res)


def kernel_traced(x, h, **kw):
    nc = build_nc()
    in_maps = _prep_core_inputs(x, h)
    res = run_bass_kernel_spmd(
        nc, in_maps, core_ids=list(range(NCORES)), trace=True, **kw
    )
    return _unstage(res), res


# revision 45
# speedup vs baseline: 1.0046x; 1.0046x over previous
"""Time-varying FIR (AllZeroDigitalFilter) on 8 TRN2 NeuronCores — v12.
~90.2us vs the 130us v3.1 baseline.

Structure (per core: 2 sequences x 8 chunks x 126 frame-rows):
  C[k, i'] = sum_j h[k, j] * x[k*80 + i' - j + pad],  i' in [0,160)
  y[k*80+i] = w1[i]*C_{k+1}[i] + w0[i]*C_k[80+i]

Key mechanism (the big win over v3.1): the free dim is INTERLEAVED —
col = u*8 + c (chunk innermost). The DVE tap-mult's x operand is then
a contiguous fp16 slice and the h-broadcast operand has innermost AP
[1, 8] (the 160-sample repeat is a stride-0 MIDDLE dim), so
tensor_tensor qualifies for the 2x_1p DVE perf mode (checked on the
innermost AP dim only: 2-byte dtype, step +-1, >=2 elems, 4B-aligned)
— 2 elem/lane/cycle, ~700ns/tap vs ~1.5us at 1x in the v3.1 layout.

Division of labor:
  DVE: 45 taps/seq as 2x-mode multi-tap mults (9 quads + 1 double +
       7 singles; extra taps ride a middle AP dim: stride -8 on x,
       +16 on h, amortizing the 58-cycle DVE init; the small-issue
       taper at the end lets the PE drain its quad-burst backlog
       before the pe_d handoff)
  ACT: 5 taps/seq as 8 chunk-wise Copy-with-scale products (strided
       step-8 APs, ~711ns each on HW — strided ScalarE ops cost ~40%
       more than contiguous; the cost model does not show this)
  PE : accumulates every plane into PSUM via identity-stationary
       matmuls (windows 480|480|320, ~541ns/plane issue rate; ldw-opt
       dedups the identity LDWEIGHTS); also applies the +1 partition
       shift (Vs = SH.T @ V) on the lower 640 cols only
  DVE tail: V = psum*rr; y = V_hi + Vs_psum (fp16 out, host casts)
GpSimd: only DMA issue; its tensor ops halve concurrent DVE
throughput (shared SBUF port).

Measured HW facts this schedule is built around:
  - per-queue DMA: ~45GB/s, ~0.8us serial issue per DMA, transfers
    from one queue overlap each other -> fan the gating loads (xa0,
    hg) out as 9 pieces over the 3 DMA-capable queues (sync, scalar
    = ACT, gpsimd); DVE compute starts ~14us (7.5us of that is fixed
    preamble)
  - PE matmul streams at 1 col/cycle with ~zero per-instr overhead
    when pipelined; PSUM-read TTs (V/blend) are 1x mode
  - seq boundary: PE may not write PSUM while DVE reads it, so PE
    restarts only after blend-0 (y_s); emitting V-0/blend-0 around
    the first seq-1 quad minimizes the exposed serialization
  - device clock throttles some runs ~1.2x; verify surprising
    timings with a rerun

Hard-won rules encoded here (from v3.1):
  - PE must never write PSUM (any bank) while DVE reads PSUM: PE is
    gated on v_s / y_s around the V and blend phases.
  - Completion semaphores ride on the last real engine instruction.
  - DMA completion semaphores: shared counters are safe only with
    full-sum thresholds (wait covers ALL DMAs that inc the counter).
  - Output staged [125, 640] fp16, de-interleaved + cast on host.

Sharding: pure data parallel across batch, 2 sequences per core.
"""

import sys

for p in ("/opt/trn_rl_repo", "/root/.axon_site/_ro/trn_rl_repo"):
    if p not in sys.path:
        sys.path.append(p)

import numpy as np
import concourse.bass as bass
import concourse.mybir as mybir
from concourse.ap import AP
from concourse.bass_utils import run_bass_kernel_spmd


def _enable_ldw_opt():
    """Dedup identical LDWEIGHTS (walrus --enable-ldw-opt): saves ~40us of
    redundant identity reloads on the PE. Idempotent monkeypatch."""
    from concourse import bass_utils as _bu

    if getattr(_bu.run_command, "_ldw_opt_patched", False):
        return
    _orig = _bu.run_command

    def _patched(cmd, **kw):
        cmd = [
            "--enable-ldw-opt=true" if c == "--enable-ldw-opt=false" else c
            for c in cmd
        ]
        return _orig(cmd, **kw)

    _patched._ldw_opt_patched = True
    _bu.run_command = _patched


_enable_ldw_opt()

B, T = 16, 80000
P, D = 80, 50
N = T // P
NCORES = 8
S = B // NCORES
FO = 125
NC = 8
CS = 212
PAD = D - 1 + P  # 129
GF = NC * 160  # 1280
XF = NC * CS  # 1696
WIN = [(0, 512), (512, 512), (1024, 256)]

F16 = mybir.dt.float16
FP32 = mybir.dt.float32

ND = 45  # DVE taps per seq
NA = 5  # ACT taps per seq: 45..49
ACT_TAPS = list(range(ND, D))
NA16 = NA * 16
# DVE issues per seq: 9 quads (taps 0..35), 1 double (36..37), then 7
# singles — the long taper lets the PE (0.54us/plane vs quad bursts of
# 4, ~2us behind after the quads) fully drain its backlog before the
# end-of-seq pe_d handoff.
NQ = 9
NISS = NQ + 8


def issue_planes(d):
    """planes (1-based l values) covered by issue d"""
    if d < NQ:
        return list(range(4 * d + 1, 4 * d + 5))
    if d == NQ:
        return [4 * NQ + 1, 4 * NQ + 2]
    return [4 * NQ + 2 + (d - NQ)]


_nc_cache = {}


def plane_schedule():
    """PE consume order. D-planes land in quad/double bursts; A-planes
    every ~5.7us; A-ready biased +2us."""

    def t_d(l):
        if l <= 4 * NQ:
            return 2729.0 * ((l + 3) // 4)
        if l <= 4 * NQ + 2:
            return 2729.0 * NQ + 1402.0
        return 2729.0 * NQ + 1402.0 + 826.0 * (l - 4 * NQ - 2)

    ev = [("D", k, t_d(k + 1)) for k in range(ND)]
    ev += [("A", k, 5700.0 * (k + 1) + 1000.0) for k in range(NA)]
    ev.sort(key=lambda e: e[2])
    return [(kind, k) for kind, k, _ in ev]


def build_nc():
    if "nc" in _nc_cache:
        return _nc_cache["nc"]
    nc = bass.Bass()
    xa_ext = nc.declare_dram_parameter("xae", [S, 126, XF], F16, isOutput=False)
    hg_ext = nc.declare_dram_parameter("hg", [126, D * 16], F16, isOutput=False)
    hg32_ext = nc.declare_dram_parameter("hg32", [126, NA16], FP32, isOutput=False)
    rr_ext = nc.declare_dram_parameter("rr", [128, 160], F16, isOutput=False)
    eye_ext = nc.declare_dram_parameter("eye", [126, 126], F16, isOutput=False)
    shm_ext = nc.declare_dram_parameter("shm", [126, 126], F16, isOutput=False)
    out_ext = nc.declare_dram_parameter("out", [S, FO, GF // 2], F16, isOutput=True)

    from contextlib import ExitStack

    with ExitStack() as _ctx:
        ec = _ctx.enter_context
        xa0 = ec(nc.sbuf_tensor([126, XF], F16))
        xa1 = ec(nc.sbuf_tensor([126, XF], F16))
        hgx = ec(nc.sbuf_tensor([126, D * 16], F16))
        hgx32 = ec(nc.sbuf_tensor([126, NA16], FP32))
        rrt = ec(nc.sbuf_tensor([128, 160], F16))
        eye = ec(nc.sbuf_tensor([126, 126], F16))
        shm = ec(nc.sbuf_tensor([126, 126], F16))
        dpb = [ec(nc.sbuf_tensor(f"dpl{i}", [126, 4 * GF], F16)) for i in range(6)]
        apb = [ec(nc.sbuf_tensor(f"apl{i}", [126, GF], F16)) for i in range(6)]
        vt0 = ec(nc.sbuf_tensor([126, GF], F16))
        vt1 = ec(nc.sbuf_tensor([126, GF], F16))
        yy0 = ec(nc.sbuf_tensor([125, GF // 2], F16))
        yy1 = ec(nc.sbuf_tensor([125, GF // 2], F16))
        ps0 = ec(nc.psum_tensor("ps0", [126, GF], FP32))
        ps1 = ec(nc.psum_tensor("ps1", [126, GF], FP32))
        xin0 = ec(nc.semaphore("xin0"))
        xin1 = ec(nc.semaphore("xin1"))
        o_s2 = ec(nc.semaphore("o_s2"))
        cin = ec(nc.semaphore("cin"))
        hgin = ec(nc.semaphore("hgin"))
        pein = ec(nc.semaphore("pein"))
        dp_s = ec(nc.semaphore("dp_s"))
        dc_s = ec(nc.semaphore("dc_s"))
        ap_s = ec(nc.semaphore("ap_s"))
        ac_s = ec(nc.semaphore("ac_s"))
        pe_d = ec(nc.semaphore("pe_d"))
        v_s = ec(nc.semaphore("v_s"))
        vsr = ec(nc.semaphore("vsr"))
        y_s = ec(nc.semaphore("y_s"))
        o_s = ec(nc.semaphore("o_s"))
        o_sg = ec(nc.semaphore("o_sg"))
        scr = ec(nc.sbuf_tensor([1, 8], F16))
        block = ec(nc.Block())

        xa = [xa0, xa1]
        xin = [xin0, xin1]
        vt = [vt0, vt1]
        yy = [yy0, yy1]
        psall = [ps0, ps1]

        def xslice(s, j):
            # contiguous interleaved window: col u*8+c, u from D-1-j
            o = (D - 1 - j) * NC
            return xa[s][:, o : o + GF]

        def xmulti(s, j, n):
            # taps j..j+n-1 as middle dim (stride -8): n contiguous
            # 1280-windows
            return AP(
                tensor=xa[s][:].tensor,
                offset=(D - 1 - j) * NC,
                ap=[[XF, 126], [-NC, n], [1, GF]],
            )

        def hb(s, j):
            # h[cFO+k, j] at free (u, c): innermost [1, 8] step-1 (2x
            # eligible), 160-repeat as stride-0 middle dim
            return AP(
                tensor=hgx[:].tensor,
                offset=j * 16 + s * NC,
                ap=[[D * 16, 126], [0, 160], [1, NC]],
            )

        def hbmulti(s, j, n):
            return AP(
                tensor=hgx[:].tensor,
                offset=j * 16 + s * NC,
                ap=[[D * 16, 126], [16, n], [0, 160], [1, NC]],
            )

        def xact(s, j, c):
            return AP(
                tensor=xa[s][:].tensor,
                offset=(D - 1 - j) * NC + c,
                ap=[[XF, 126], [NC, 160]],
            )

        def rrb(o, ln):
            return AP(
                tensor=rrt[:].tensor,
                offset=o // NC,
                ap=[[160, 126], [1, ln // NC], [0, NC]],
            )

        # input DMA fan-out (only sync/scalar/gpsimd queues can DMA).
        # Small pieces pipeline within a queue (~0.8us issue, ~45GB/s
        # per transfer, transfers overlap), so 2 xa0 pieces + 1 hg piece
        # per queue beats one big DMA per queue.
        @block.sync
        def _(sync):
            sync.dma_start(xa[0][84:105, :], xa_ext[0, 84:105, :]).then_inc(
                xin0, 16
            )
            sync.dma_start(xa[0][105:126, :], xa_ext[0, 105:126, :]).then_inc(
                xin0, 16
            )
            sync.dma_start(hgx[84:126, :], hg_ext[84:126, :]).then_inc(hgin, 16)
            sync.dma_start(eye[:], eye_ext[:]).then_inc(pein, 16)
            sync.dma_start(shm[:], shm_ext[:]).then_inc(pein, 16)
            sync.dma_start(xa[1][0:63, :], xa_ext[1, 0:63, :]).then_inc(xin1, 16)
            sync.dma_start(xa[1][63:126, :], xa_ext[1, 63:126, :]).then_inc(
                xin1, 16
            )
            sync.wait_ge(y_s, 1)
            sync.dma_start(out_ext[0, :, :], yy[0][:]).then_inc(o_s, 16)
            sync.wait_ge(y_s, 2)
            sync.dma_start(out_ext[1, :, 0:240], yy[1][:, 0:240]).then_inc(o_s, 16)
            sync.wait_ge(o_s, 32)
            sync.wait_ge(o_s2, 16)
            sync.wait_ge(o_sg, 16)

        @block.gpsimd
        def _(gpsimd):
            gpsimd.dma_start(xa[0][42:63, :], xa_ext[0, 42:63, :]).then_inc(
                xin0, 16
            )
            gpsimd.dma_start(xa[0][63:84, :], xa_ext[0, 63:84, :]).then_inc(
                xin0, 16
            )
            gpsimd.dma_start(hgx[42:84, :], hg_ext[42:84, :]).then_inc(hgin, 16)
            gpsimd.wait_ge(y_s, 2)
            gpsimd.dma_start(
                out_ext[1, :, 240:480], yy[1][:, 240:480]
            ).then_inc(o_sg, 16)

        def emit_vmult(vector, s):
            if s == 0:
                vector.wait_ge(cin, 32)
            vector.wait_ge(pe_d, s + 1)
            inst = vector.tensor_tensor(
                out=vt[s][:],
                in0=psall[s][:],
                in1=rrb(0, GF),
                op=mybir.AluOpType.mult,
            )
            inst.then_inc(v_s, 1)

        def emit_yblend(vector, s):
            # y[q, (i,c)] = V[q, (80+i,c)] + Vs[q, (i,c)]; Vs lives in
            # psum cols 0:640 (the DVE read spans bank 0 + half of 1)
            vector.wait_ge(vsr, s + 1)
            inst = vector.tensor_tensor(
                out=yy[s][:],
                in0=vt[s][0:125, 640:1280],
                in1=psall[s][0:125, 0:640],
                op=mybir.AluOpType.add,
            )
            inst.then_inc(y_s, 1)

        @block.vector
        def _(vector):
            vector.wait_ge(hgin, 48)
            for s in range(S):
                vector.wait_ge(xin[s], 96 if s == 0 else 32)
                for d in range(NISS):
                    i_glob = s * NISS + d
                    if i_glob >= 6:
                        vector.wait_ge(dc_s, i_glob - 5)
                    buf = dpb[i_glob % 6]
                    pl = issue_planes(d)
                    j = pl[0] - 1
                    if len(pl) == 1:
                        inst = vector.tensor_tensor(
                            out=buf[:, 0:GF],
                            in0=hb(s, j),
                            in1=xslice(s, j),
                            op=mybir.AluOpType.mult,
                        )
                    else:
                        inst = vector.tensor_tensor(
                            out=buf[:, 0 : len(pl) * GF],
                            in0=hbmulti(s, j, len(pl)),
                            in1=xmulti(s, j, len(pl)),
                            op=mybir.AluOpType.mult,
                        )
                    inst.then_inc(dp_s, len(pl))
                    if s == 1 and d == 0:
                        # seq-0 V after the first seq-1 quad (pe_d=1
                        # lands during that quad: no DVE stall), and
                        # blend-0 right behind it (~0.6us vsr stall)
                        # so y_s unblocks the PE's seq-1 asap — the
                        # pipeline end is PE-consumption-bound
                        emit_vmult(vector, 0)
                        emit_yblend(vector, 0)
            emit_vmult(vector, 1)
            emit_yblend(vector, 1)

        @block.tensor
        def _(tensor):
            tensor.wait_ge(pein, 32)
            sched = plane_schedule()
            nplanes = ND + NA
            for s in range(S):
                # No gate before seq-1: its planes write ps1 while the
                # DVE's blend-0 reads ps0 — testing that the PSUM
                # write-while-read hazard is bank-level, not global.
                # (Program order already puts shift-0, and thus
                # vmult-0's ps0 read, before any ps1 write.)
                done = 0
                for kind, k in sched:
                    if kind == "D":
                        l = k + 1  # plane index within seq, 1..45
                        g = s * ND + l
                        tensor.wait_ge(dp_s, g)
                        if l <= 4 * NQ:
                            d, off = (l - 1) // 4, (l - 1) % 4
                        elif l <= 4 * NQ + 2:
                            d, off = NQ, l - 4 * NQ - 1
                        else:
                            d, off = NQ + (l - 4 * NQ - 2), 0
                        last_of_issue = l == issue_planes(d)[-1]
                        src = dpb[(s * NISS + d) % 6]
                        base = off * GF
                    else:
                        g = s * NA + k + 1
                        tensor.wait_ge(ap_s, g)
                        src = apb[(g - 1) % 6]
                        base = 0
                    first = done == 0
                    done += 1
                    last = done == nplanes
                    for w, (o, ln) in enumerate(WIN):
                        inst = tensor.matmul(
                            psall[s][:, o : o + ln],
                            eye[:],
                            src[:, base + o : base + o + ln],
                            start=first,
                            stop=last,
                            skip_group_check=True,
                        )
                    if done == nplanes:
                        inst.then_inc(pe_d, 1)
                    elif kind == "D" and last_of_issue:
                        inst.then_inc(dc_s, 1)
                    elif kind == "A" and g % 2 == 0:
                        inst.then_inc(ac_s, 1)
                # Vs = SH.T @ V (the +1 partition shift) on the lower 640
                # cols only (all the blend reads), reusing this sequence's
                # psum banks after the V-mult consumed them
                tensor.wait_ge(v_s, s + 1)
                tensor.matmul(
                    psall[s][:, 0:512],
                    shm[:],
                    vt[s][:, 0:512],
                    start=True,
                    stop=True,
                    skip_group_check=True,
                )
                inst = tensor.matmul(
                    psall[s][:, 512:640],
                    shm[:],
                    vt[s][:, 512:640],
                    start=True,
                    stop=True,
                    skip_group_check=True,
                )
                inst.then_inc(vsr, 1)

        @block.scalar
        def _(scalar):
            scalar.dma_start(xa[0][0:21, :], xa_ext[0, 0:21, :]).then_inc(
                xin0, 16
            )
            scalar.dma_start(xa[0][21:42, :], xa_ext[0, 21:42, :]).then_inc(
                xin0, 16
            )
            scalar.dma_start(hgx[0:42, :], hg_ext[0:42, :]).then_inc(hgin, 16)
            scalar.dma_start(rrt[:], rr_ext[:]).then_inc(cin, 16)
            scalar.dma_start(hgx32[:], hg32_ext[:]).then_inc(cin, 16)
            # dummy activation pulls the ~1.3us ACT_TABLE_LOAD ahead of
            # the first real tap product
            scalar.activation(
                scr[:, 4:8], scr[:, 0:4], mybir.ActivationFunctionType.Copy
            )
            scalar.wait_ge(cin, 32)
            for s in range(S):
                scalar.wait_ge(xin[s], 96 if s == 0 else 32)
                for kk, j in enumerate(ACT_TAPS):
                    g = s * NA + kk + 1
                    if g > 6:
                        scalar.wait_ge(ac_s, (g - 6 + 1) // 2)
                    for c in range(NC):
                        inst = scalar.activation(
                            AP(
                                tensor=apb[(g - 1) % 6][:].tensor,
                                offset=c,
                                ap=[[GF, 126], [NC, 160]],
                            ),
                            xact(s, j, c),
                            mybir.ActivationFunctionType.Copy,
                            scale=hgx32[
                                :, kk * 16 + s * NC + c : kk * 16 + s * NC + c + 1
                            ],
                        )
                    inst.then_inc(ap_s, 1)
            scalar.wait_ge(y_s, 2)
            scalar.dma_start(out_ext[1, :, 480:640], yy[1][:, 480:640]).then_inc(
                o_s2, 16
            )

    _nc_cache["nc"] = nc
    return nc


def _prep_core_inputs(x, h):
    x = np.ascontiguousarray(x, dtype=np.float32)
    h = np.ascontiguousarray(h, dtype=np.float32)
    TPX = N * P + CS + 4
    xp = np.zeros((B, TPX), np.float16)
    xp[:, PAD : PAD + T] = x.astype(np.float16)
    # host im2col, interleaved: xae[b, q, u*8+c] = xp[b, (c*FO + q)*P + u]
    idx_q = (np.arange(126)[:, None, None] * P + np.arange(CS)[None, :, None]
             + np.arange(NC)[None, None, :] * FO * P)  # [126, CS, NC]
    xae_all = xp[:, idx_q.reshape(-1)].reshape(B, 126, NC * CS)
    hpad = np.concatenate([h, h[:, -1:, :]], axis=1)
    w1 = (np.arange(P, dtype=np.float32) / P).astype(np.float16)
    w0 = (1.0 - np.arange(P, dtype=np.float32) / P).astype(np.float16)
    rr = np.ascontiguousarray(
        np.broadcast_to(np.concatenate([w1, w0])[None, :], (128, 160))
    )
    eye = np.eye(126, dtype=np.float16)
    shm = np.zeros((126, 126), np.float16)
    shm[np.arange(1, 126), np.arange(125)] = 1.0  # SH[k', m] = 1 iff k' == m+1
    in_maps = []
    acols = np.concatenate([np.arange(j * 16, j * 16 + 16) for j in ACT_TAPS])
    for core in range(NCORES):
        sl = slice(core * S, (core + 1) * S)
        hc = hpad[sl]
        hg32 = np.zeros((126, D * 16), np.float32)
        for s in range(S):
            for c in range(NC):
                blk = hc[s, c * FO : c * FO + 126, :]
                hg32[:, s * NC + c :: 16] = blk
        in_maps.append(
            {
                "xae": np.ascontiguousarray(xae_all[sl]),
                "hg": hg32.astype(np.float16),
                "hg32": np.ascontiguousarray(hg32[:, acols]),
                "rr": rr,
                "eye": eye,
                "shm": shm,
            }
        )
    return in_maps


def _unstage(res):
    outs = []
    for c in range(NCORES):
        o = res.results[c]["out"]  # [S, 125, 80*8] fp16, col = i*8+c
        o = o.reshape(S, FO, P, NC).transpose(0, 3, 1, 2).reshape(S, T)
        outs.append(o)
    return np.ascontiguousarray(
        np.concatenate(outs, axis=0), dtype=np.float32
    )


def kernel(x, h, **kw):
    nc = build_nc()
    in_maps = _prep_core_inputs(x, h)
    res = run_bass_kernel_spmd(nc, in_maps, core_ids=list(range(NCORES)), **kw)
    return _unstage(res)


def kernel_traced(x, h, **kw):
    nc = build_nc()
    in_maps = _prep_core_inputs(x, h)
    res = run_bass_kernel_spmd(
        nc, in_maps, core_ids=list(range(NCORES)), trace=True, **kw
    )
    return _unstage(res), res


# revision 46
# speedup vs baseline: 1.0198x; 1.0151x over previous
"""Time-varying FIR (AllZeroDigitalFilter) on 8 TRN2 NeuronCores — v12.
~90.2us vs the 130us v3.1 baseline.

Structure (per core: 2 sequences x 8 chunks x 126 frame-rows):
  C[k, i'] = sum_j h[k, j] * x[k*80 + i' - j + pad],  i' in [0,160)
  y[k*80+i] = w1[i]*C_{k+1}[i] + w0[i]*C_k[80+i]

Key mechanism (the big win over v3.1): the free dim is INTERLEAVED —
col = u*8 + c (chunk innermost). The DVE tap-mult's x operand is then
a contiguous fp16 slice and the h-broadcast operand has innermost AP
[1, 8] (the 160-sample repeat is a stride-0 MIDDLE dim), so
tensor_tensor qualifies for the 2x_1p DVE perf mode (checked on the
innermost AP dim only: 2-byte dtype, step +-1, >=2 elems, 4B-aligned)
— 2 elem/lane/cycle, ~700ns/tap vs ~1.5us at 1x in the v3.1 layout.

Division of labor:
  DVE: 45 taps/seq as 2x-mode multi-tap mults (9 quads + 1 double +
       7 singles; extra taps ride a middle AP dim: stride -8 on x,
       +16 on h, amortizing the 58-cycle DVE init; the small-issue
       taper at the end lets the PE drain its quad-burst backlog
       before the pe_d handoff)
  ACT: 5 taps/seq as 8 chunk-wise Copy-with-scale products (strided
       step-8 APs, ~711ns each on HW — strided ScalarE ops cost ~40%
       more than contiguous; the cost model does not show this)
  PE : accumulates every plane into PSUM via identity-stationary
       matmuls (windows 480|480|320, ~541ns/plane issue rate; ldw-opt
       dedups the identity LDWEIGHTS); also applies the +1 partition
       shift (Vs = SH.T @ V) on the lower 640 cols only
  DVE tail: V = psum*rr; y = V_hi + Vs_psum (fp16 out, host casts)
GpSimd: only DMA issue; its tensor ops halve concurrent DVE
throughput (shared SBUF port).

Measured HW facts this schedule is built around:
  - per-queue DMA: ~45GB/s, ~0.8us serial issue per DMA, transfers
    from one queue overlap each other -> fan the gating loads (xa0,
    hg) out as 9 pieces over the 3 DMA-capable queues (sync, scalar
    = ACT, gpsimd); DVE compute starts ~14us (7.5us of that is fixed
    preamble)
  - PE matmul streams at 1 col/cycle with ~zero per-instr overhead
    when pipelined; PSUM-read TTs (V/blend) are 1x mode
  - seq boundary: PE may not write PSUM while DVE reads it, so PE
    restarts only after blend-0 (y_s); emitting V-0/blend-0 around
    the first seq-1 quad minimizes the exposed serialization
  - device clock throttles some runs ~1.2x; verify surprising
    timings with a rerun

Hard-won rules encoded here (from v3.1):
  - PE must never write PSUM (any bank) while DVE reads PSUM: PE is
    gated on v_s / y_s around the V and blend phases.
  - Completion semaphores ride on the last real engine instruction.
  - DMA completion semaphores: shared counters are safe only with
    full-sum thresholds (wait covers ALL DMAs that inc the counter).
  - Output staged [125, 640] fp16, de-interleaved + cast on host.

Sharding: pure data parallel across batch, 2 sequences per core.
"""

import sys

for p in ("/opt/trn_rl_repo", "/root/.axon_site/_ro/trn_rl_repo"):
    if p not in sys.path:
        sys.path.append(p)

import numpy as np
import concourse.bass as bass
import concourse.mybir as mybir
from concourse.ap import AP
from concourse.bass_utils import run_bass_kernel_spmd


def _enable_ldw_opt():
    """Dedup identical LDWEIGHTS (walrus --enable-ldw-opt): saves ~40us of
    redundant identity reloads on the PE. Idempotent monkeypatch."""
    from concourse import bass_utils as _bu

    if getattr(_bu.run_command, "_ldw_opt_patched", False):
        return
    _orig = _bu.run_command

    def _patched(cmd, **kw):
        cmd = [
            "--enable-ldw-opt=true" if c == "--enable-ldw-opt=false" else c
            for c in cmd
        ]
        return _orig(cmd, **kw)

    _patched._ldw_opt_patched = True
    _bu.run_command = _patched


_enable_ldw_opt()

B, T = 16, 80000
P, D = 80, 50
N = T // P
NCORES = 8
S = B // NCORES
FO = 125
NC = 8
CS = 212
PAD = D - 1 + P  # 129
GF = NC * 160  # 1280
XF = NC * CS  # 1696
WIN = [(0, 512), (512, 512), (1024, 256)]

F16 = mybir.dt.float16
FP32 = mybir.dt.float32

ND = 45  # DVE taps per seq
NA = 5  # ACT taps per seq: 45..49
ACT_TAPS = list(range(ND, D))
NA16 = NA * 16
# DVE issues per seq: 9 quads (taps 0..35), 1 double (36..37), then 7
# singles — the long taper lets the PE (0.54us/plane vs quad bursts of
# 4, ~2us behind after the quads) fully drain its backlog before the
# end-of-seq pe_d handoff.
NQ = 9
NISS = NQ + 8


def issue_planes(d):
    """planes (1-based l values) covered by issue d"""
    if d < NQ:
        return list(range(4 * d + 1, 4 * d + 5))
    if d == NQ:
        return [4 * NQ + 1, 4 * NQ + 2]
    return [4 * NQ + 2 + (d - NQ)]


_nc_cache = {}


def plane_schedule():
    """PE consume order. D-planes land in quad/double bursts; A-planes
    every ~5.7us; A-ready biased +2us."""

    def t_d(l):
        if l <= 4 * NQ:
            return 2729.0 * ((l + 3) // 4)
        if l <= 4 * NQ + 2:
            return 2729.0 * NQ + 1402.0
        return 2729.0 * NQ + 1402.0 + 826.0 * (l - 4 * NQ - 2)

    ev = [("D", k, t_d(k + 1)) for k in range(ND)]
    ev += [("A", k, 5700.0 * (k + 1) + 1000.0) for k in range(NA)]
    ev.sort(key=lambda e: e[2])
    return [(kind, k) for kind, k, _ in ev]


def build_nc():
    if "nc" in _nc_cache:
        return _nc_cache["nc"]
    nc = bass.Bass()
    xa_ext = nc.declare_dram_parameter("xae", [S, 126, XF], F16, isOutput=False)
    hg_ext = nc.declare_dram_parameter("hg", [126, D * 16], F16, isOutput=False)
    hg32_ext = nc.declare_dram_parameter("hg32", [126, NA16], FP32, isOutput=False)
    rr_ext = nc.declare_dram_parameter("rr", [128, 160], F16, isOutput=False)
    eye_ext = nc.declare_dram_parameter("eye", [126, 126], F16, isOutput=False)
    shm_ext = nc.declare_dram_parameter("shm", [126, 126], F16, isOutput=False)
    out_ext = nc.declare_dram_parameter("out", [S, FO, GF // 2], F16, isOutput=True)

    from contextlib import ExitStack

    with ExitStack() as _ctx:
        ec = _ctx.enter_context
        xa0 = ec(nc.sbuf_tensor([126, XF], F16))
        xa1 = ec(nc.sbuf_tensor([126, XF], F16))
        hgx = ec(nc.sbuf_tensor([126, D * 16], F16))
        hgx32 = ec(nc.sbuf_tensor([126, NA16], FP32))
        rrt = ec(nc.sbuf_tensor([128, 160], F16))
        eye = ec(nc.sbuf_tensor([126, 126], F16))
        shm = ec(nc.sbuf_tensor([126, 126], F16))
        dpb = [ec(nc.sbuf_tensor(f"dpl{i}", [126, 4 * GF], F16)) for i in range(6)]
        apb = [ec(nc.sbuf_tensor(f"apl{i}", [126, GF], F16)) for i in range(6)]
        vt0 = ec(nc.sbuf_tensor([126, GF], F16))
        vt1 = ec(nc.sbuf_tensor([126, GF], F16))
        yy0 = ec(nc.sbuf_tensor([125, GF // 2], F16))
        yy1 = ec(nc.sbuf_tensor([125, GF // 2], F16))
        ps0 = ec(nc.psum_tensor("ps0", [126, GF], FP32))
        ps1 = ec(nc.psum_tensor("ps1", [126, GF], FP32))
        xin0 = ec(nc.semaphore("xin0"))
        xin1 = ec(nc.semaphore("xin1"))
        o_s2 = ec(nc.semaphore("o_s2"))
        cin = ec(nc.semaphore("cin"))
        hgin = ec(nc.semaphore("hgin"))
        pein = ec(nc.semaphore("pein"))
        dp_s = ec(nc.semaphore("dp_s"))
        dc_s = ec(nc.semaphore("dc_s"))
        ap_s = ec(nc.semaphore("ap_s"))
        ac_s = ec(nc.semaphore("ac_s"))
        pe_d = ec(nc.semaphore("pe_d"))
        v_s = ec(nc.semaphore("v_s"))
        vsr = ec(nc.semaphore("vsr"))
        y_s = ec(nc.semaphore("y_s"))
        o_s = ec(nc.semaphore("o_s"))
        o_sg = ec(nc.semaphore("o_sg"))
        scr = ec(nc.sbuf_tensor([1, 8], F16))
        block = ec(nc.Block())

        xa = [xa0, xa1]
        xin = [xin0, xin1]
        vt = [vt0, vt1]
        yy = [yy0, yy1]
        psall = [ps0, ps1]

        def xslice(s, j):
            # contiguous interleaved window: col u*8+c, u from D-1-j
            o = (D - 1 - j) * NC
            return xa[s][:, o : o + GF]

        def xmulti(s, j, n):
            # taps j..j+n-1 as middle dim (stride -8): n contiguous
            # 1280-windows
            return AP(
                tensor=xa[s][:].tensor,
                offset=(D - 1 - j) * NC,
                ap=[[XF, 126], [-NC, n], [1, GF]],
            )

        def hb(s, j):
            # h[cFO+k, j] at free (u, c): innermost [1, 8] step-1 (2x
            # eligible), 160-repeat as stride-0 middle dim
            return AP(
                tensor=hgx[:].tensor,
                offset=j * 16 + s * NC,
                ap=[[D * 16, 126], [0, 160], [1, NC]],
            )

        def hbmulti(s, j, n):
            return AP(
                tensor=hgx[:].tensor,
                offset=j * 16 + s * NC,
                ap=[[D * 16, 126], [16, n], [0, 160], [1, NC]],
            )

        def xact(s, j, c):
            return AP(
                tensor=xa[s][:].tensor,
                offset=(D - 1 - j) * NC + c,
                ap=[[XF, 126], [NC, 160]],
            )

        def rrb(o, ln):
            return AP(
                tensor=rrt[:].tensor,
                offset=o // NC,
                ap=[[160, 126], [1, ln // NC], [0, NC]],
            )

        # input DMA fan-out (only sync/scalar/gpsimd queues can DMA).
        # Small pieces pipeline within a queue (~0.8us issue, ~45GB/s
        # per transfer, transfers overlap), so 2 xa0 pieces + 1 hg piece
        # per queue beats one big DMA per queue.
        @block.sync
        def _(sync):
            sync.dma_start(xa[0][84:105, :], xa_ext[0, 84:105, :]).then_inc(
                xin0, 16
            )
            sync.dma_start(xa[0][105:126, :], xa_ext[0, 105:126, :]).then_inc(
                xin0, 16
            )
            sync.dma_start(hgx[84:126, :], hg_ext[84:126, :]).then_inc(hgin, 16)
            sync.dma_start(eye[:], eye_ext[:]).then_inc(pein, 16)
            sync.dma_start(shm[:], shm_ext[:]).then_inc(pein, 16)
            sync.dma_start(xa[1][0:63, :], xa_ext[1, 0:63, :]).then_inc(xin1, 16)
            sync.dma_start(xa[1][63:126, :], xa_ext[1, 63:126, :]).then_inc(
                xin1, 16
            )
            sync.wait_ge(y_s, 1)
            sync.dma_start(out_ext[0, :, :], yy[0][:]).then_inc(o_s, 16)
            sync.wait_ge(y_s, 2)
            sync.dma_start(out_ext[1, :, 0:240], yy[1][:, 0:240]).then_inc(o_s, 16)
            sync.wait_ge(o_s, 32)
            sync.wait_ge(o_s2, 16)
            sync.wait_ge(o_sg, 16)

        @block.gpsimd
        def _(gpsimd):
            gpsimd.dma_start(xa[0][42:63, :], xa_ext[0, 42:63, :]).then_inc(
                xin0, 16
            )
            gpsimd.dma_start(xa[0][63:84, :], xa_ext[0, 63:84, :]).then_inc(
                xin0, 16
            )
            gpsimd.dma_start(hgx[42:84, :], hg_ext[42:84, :]).then_inc(hgin, 16)
            gpsimd.wait_ge(y_s, 2)
            gpsimd.dma_start(
                out_ext[1, :, 240:480], yy[1][:, 240:480]
            ).then_inc(o_sg, 16)

        def emit_vmult(vector, s):
            if s == 0:
                vector.wait_ge(cin, 32)
            vector.wait_ge(pe_d, s + 1)
            inst = vector.tensor_tensor(
                out=vt[s][:],
                in0=psall[s][:],
                in1=rrb(0, GF),
                op=mybir.AluOpType.mult,
            )
            inst.then_inc(v_s, 1)

        def emit_yblend(vector, s):
            # y[q, (i,c)] = V[q, (80+i,c)] + Vs[q, (i,c)]; Vs lives in
            # psum cols 0:640 (the DVE read spans bank 0 + half of 1)
            vector.wait_ge(vsr, s + 1)
            inst = vector.tensor_tensor(
                out=yy[s][:],
                in0=vt[s][0:125, 640:1280],
                in1=psall[s][0:125, 0:640],
                op=mybir.AluOpType.add,
            )
            inst.then_inc(y_s, 1)

        @block.vector
        def _(vector):
            vector.wait_ge(hgin, 48)
            for s in range(S):
                vector.wait_ge(xin[s], 96 if s == 0 else 32)
                for d in range(NISS):
                    i_glob = s * NISS + d
                    if i_glob >= 6:
                        vector.wait_ge(dc_s, i_glob - 5)
                    buf = dpb[i_glob % 6]
                    pl = issue_planes(d)
                    j = pl[0] - 1
                    if len(pl) == 1:
                        inst = vector.tensor_tensor(
                            out=buf[:, 0:GF],
                            in0=hb(s, j),
                            in1=xslice(s, j),
                            op=mybir.AluOpType.mult,
                        )
                    else:
                        inst = vector.tensor_tensor(
                            out=buf[:, 0 : len(pl) * GF],
                            in0=hbmulti(s, j, len(pl)),
                            in1=xmulti(s, j, len(pl)),
                            op=mybir.AluOpType.mult,
                        )
                    inst.then_inc(dp_s, len(pl))
                    if s == 1 and d == 0:
                        # seq-0 V after the first seq-1 quad (pe_d=1
                        # lands during that quad: no DVE stall)
                        emit_vmult(vector, 0)
                    if s == 1 and d == 2:
                        # blend-0 two quads later: vsr (PE shift-0) is
                        # long done, no stall; nothing downstream is
                        # gated on y_s anymore except the out0 DMA
                        emit_yblend(vector, 0)
            emit_vmult(vector, 1)
            emit_yblend(vector, 1)

        @block.tensor
        def _(tensor):
            tensor.wait_ge(pein, 32)
            sched = plane_schedule()
            nplanes = ND + NA
            for s in range(S):
                # No gate before seq-1: its planes write ps1 while the
                # DVE's blend-0 reads ps0 — testing that the PSUM
                # write-while-read hazard is bank-level, not global.
                # (Program order already puts shift-0, and thus
                # vmult-0's ps0 read, before any ps1 write.)
                done = 0
                for kind, k in sched:
                    if kind == "D":
                        l = k + 1  # plane index within seq, 1..45
                        g = s * ND + l
                        tensor.wait_ge(dp_s, g)
                        if l <= 4 * NQ:
                            d, off = (l - 1) // 4, (l - 1) % 4
                        elif l <= 4 * NQ + 2:
                            d, off = NQ, l - 4 * NQ - 1
                        else:
                            d, off = NQ + (l - 4 * NQ - 2), 0
                        last_of_issue = l == issue_planes(d)[-1]
                        src = dpb[(s * NISS + d) % 6]
                        base = off * GF
                    else:
                        g = s * NA + k + 1
                        tensor.wait_ge(ap_s, g)
                        src = apb[(g - 1) % 6]
                        base = 0
                    first = done == 0
                    done += 1
                    last = done == nplanes
                    for w, (o, ln) in enumerate(WIN):
                        inst = tensor.matmul(
                            psall[s][:, o : o + ln],
                            eye[:],
                            src[:, base + o : base + o + ln],
                            start=first,
                            stop=last,
                            skip_group_check=True,
                        )
                    if done == nplanes:
                        inst.then_inc(pe_d, 1)
                    elif kind == "D" and last_of_issue:
                        inst.then_inc(dc_s, 1)
                    elif kind == "A" and g % 2 == 0:
                        inst.then_inc(ac_s, 1)
                # Vs = SH.T @ V (the +1 partition shift) on the lower 640
                # cols only (all the blend reads), reusing this sequence's
                # psum banks after the V-mult consumed them
                tensor.wait_ge(v_s, s + 1)
                tensor.matmul(
                    psall[s][:, 0:512],
                    shm[:],
                    vt[s][:, 0:512],
                    start=True,
                    stop=True,
                    skip_group_check=True,
                )
                inst = tensor.matmul(
                    psall[s][:, 512:640],
                    shm[:],
                    vt[s][:, 512:640],
                    start=True,
                    stop=True,
                    skip_group_check=True,
                )
                inst.then_inc(vsr, 1)

        @block.scalar
        def _(scalar):
            scalar.dma_start(xa[0][0:21, :], xa_ext[0, 0:21, :]).then_inc(
                xin0, 16
            )
            scalar.dma_start(xa[0][21:42, :], xa_ext[0, 21:42, :]).then_inc(
                xin0, 16
            )
            scalar.dma_start(hgx[0:42, :], hg_ext[0:42, :]).then_inc(hgin, 16)
            scalar.dma_start(rrt[:], rr_ext[:]).then_inc(cin, 16)
            scalar.dma_start(hgx32[:], hg32_ext[:]).then_inc(cin, 16)
            # dummy activation pulls the ~1.3us ACT_TABLE_LOAD ahead of
            # the first real tap product
            scalar.activation(
                scr[:, 4:8], scr[:, 0:4], mybir.ActivationFunctionType.Copy
            )
            scalar.wait_ge(cin, 32)
            for s in range(S):
                scalar.wait_ge(xin[s], 96 if s == 0 else 32)
                for kk, j in enumerate(ACT_TAPS):
                    g = s * NA + kk + 1
                    if g > 6:
                        scalar.wait_ge(ac_s, (g - 6 + 1) // 2)
                    for c in range(NC):
                        inst = scalar.activation(
                            AP(
                                tensor=apb[(g - 1) % 6][:].tensor,
                                offset=c,
                                ap=[[GF, 126], [NC, 160]],
                            ),
                            xact(s, j, c),
                            mybir.ActivationFunctionType.Copy,
                            scale=hgx32[
                                :, kk * 16 + s * NC + c : kk * 16 + s * NC + c + 1
                            ],
                        )
                    inst.then_inc(ap_s, 1)
            scalar.wait_ge(y_s, 2)
            scalar.dma_start(out_ext[1, :, 480:640], yy[1][:, 480:640]).then_inc(
                o_s2, 16
            )

    _nc_cache["nc"] = nc
    return nc


def _prep_core_inputs(x, h):
    x = np.ascontiguousarray(x, dtype=np.float32)
    h = np.ascontiguousarray(h, dtype=np.float32)
    TPX = N * P + CS + 4
    xp = np.zeros((B, TPX), np.float16)
    xp[:, PAD : PAD + T] = x.astype(np.float16)
    # host im2col, interleaved: xae[b, q, u*8+c] = xp[b, (c*FO + q)*P + u]
    idx_q = (np.arange(126)[:, None, None] * P + np.arange(CS)[None, :, None]
             + np.arange(NC)[None, None, :] * FO * P)  # [126, CS, NC]
    xae_all = xp[:, idx_q.reshape(-1)].reshape(B, 126, NC * CS)
    hpad = np.concatenate([h, h[:, -1:, :]], axis=1)
    w1 = (np.arange(P, dtype=np.float32) / P).astype(np.float16)
    w0 = (1.0 - np.arange(P, dtype=np.float32) / P).astype(np.float16)
    rr = np.ascontiguousarray(
        np.broadcast_to(np.concatenate([w1, w0])[None, :], (128, 160))
    )
    eye = np.eye(126, dtype=np.float16)
    shm = np.zeros((126, 126), np.float16)
    shm[np.arange(1, 126), np.arange(125)] = 1.0  # SH[k', m] = 1 iff k' == m+1
    in_maps = []
    acols = np.concatenate([np.arange(j * 16, j * 16 + 16) for j in ACT_TAPS])
    for core in range(NCORES):
        sl = slice(core * S, (core + 1) * S)
        hc = hpad[sl]
        hg32 = np.zeros((126, D * 16), np.float32)
        for s in range(S):
            for c in range(NC):
                blk = hc[s, c * FO : c * FO + 126, :]
                hg32[:, s * NC + c :: 16] = blk
        in_maps.append(
            {
                "xae": np.ascontiguousarray(xae_all[sl]),
                "hg": hg32.astype(np.float16),
                "hg32": np.ascontiguousarray(hg32[:, acols]),
                "rr": rr,
                "eye": eye,
                "shm": shm,
            }
        )
    return in_maps


def _unstage(res):
    outs = []
    for c in range(NCORES):
        o = res.results[c]["out"]  # [S, 125, 80*8] fp16, col = i*8+c
        o = o.reshape(S, FO, P, NC).transpose(0, 3, 1, 2).reshape(S, T)
        outs.append(o)
    return np.ascontiguousarray(
        np.concatenate(outs, axis=0), dtype=np.float32
    )


def kernel(x, h, **kw):
    nc = build_nc()
    in_maps = _prep_core_inputs(x, h)
    res = run_bass_kernel_spmd(nc, in_maps, core_ids=list(range(NCORES)), **kw)
    return _unstage(res)


def kernel_traced(x, h, **kw):
    nc = build_nc()
    in_maps = _prep_core_inputs(x, h)
    res = run_bass_kernel_spmd(
        nc, in_maps, core_ids=list(range(NCORES)), trace=True, **kw
    )
    return _unstage(res), res
